# revision 6
# baseline (speedup 1.0000x reference)
"""CTC loss (reduction='none', zero_infinity=True) on 8 Trainium2 NeuronCores.

Data parallel over the batch: core k owns samples [64k, 64k+64).

Per core the CTC forward recursion runs in the LINEAR probability domain with
per-step rescaling (the applied rescale reciprocals are archived exactly, so a
host f64 post-pass reconstructs the log-domain loss):

  lattice right-aligned per sample (shift by delta = S - target_len) so the two
  final lattice states sit at fixed columns for every sample; blank states B_j
  and label states Lb_j are separate arrays; W_j = skip_mask_{j+1} * Lb_j.

  per step t:
    X_j = B_j + Lb_{j-1};  U_j = B_j + W_{j-1}    (one fused op; accum_out c)
    snaps[:, t] = X_S                              (pair-sum of final states)
    rs[:, t-1] = 1/c                               (vector reciprocal)
    Y_j = Lb_j + U_j
    Lb' = (Y*r)*pl ; W' = (Y*r)*plk                (one fused two-region op)
    B' = (X*pb)*r

Emissions [pl|plk|pb] come from per-sample one-hot matmuls on the TensorEngine
(one-hot built host-side; probs = exp(log_probs) on the ScalarEngine, class-
major via an xbar DMA transpose), re-laid out to (sample, time-major) tiles by
small SBUF->SBUF DMAs, double-buffered ahead of the VectorEngine recursion.

Host post-pass: loss = -(log snaps[n, t*+1] - sum_{u<=t*} log r_u) with
t* = input_len-1, then the zero_infinity rule.
"""
import numpy as np
import ml_dtypes

import concourse.bass as bass
from concourse import mybir
from concourse.bass_utils import run_bass_kernel_spmd

T, N, C, S = 512, 512, 80, 128
NCORES = 8
NL = N // NCORES          # samples per core
CP = 128                  # classes padded to 128 (one-hot rows >= C are zero)
SS = 2 * S + 1            # emission cols per t: [pl(128) | plk(128) | pb(1)]
CH = 32                   # recursion steps per pipelined chunk
NCH = T // CH
NMM = NCH * NL            # total per-sample matmuls

_f32 = mybir.dt.float32
_bf16 = mybir.dt.bfloat16
_fp8 = mybir.dt.float8e4
_ADD = mybir.AluOpType.add
_MUL = mybir.AluOpType.mult
_EXP = mybir.ActivationFunctionType.Exp

_prog_cache = None


def _build_program():
    nc = bass.Bass()
    lp_in = nc.declare_dram_parameter("lp", [T, NL * C], _f32, isOutput=False)
    oh_in = nc.declare_dram_parameter("oh", [CP, NL * SS], _fp8, isOutput=False)
    mk_in = nc.declare_dram_parameter("masks", [NL, SS], _f32, isOutput=False)
    snaps_out = nc.declare_dram_parameter("snaps", [NL, T + 1], _f32, isOutput=True)
    rs_out = nc.declare_dram_parameter("rs", [NL, T], _f32, isOutput=True)
    scratch = nc.dram_tensor("scratch", [T * NL, CP], _bf16)

    from contextlib import ExitStack
    with ExitStack() as es:
        pcT = es.enter_context(nc.sbuf_tensor([CP, T * NL], _bf16))
        oh_t = es.enter_context(nc.sbuf_tensor([CP, NL * SS], _fp8))
        masks = es.enter_context(nc.sbuf_tensor([NL, SS], _f32))
        nat0 = es.enter_context(nc.sbuf_tensor([128, NL * C], _f32))
        pad0 = es.enter_context(nc.sbuf_tensor([128, NL * CP], _bf16))
        emit0 = es.enter_context(nc.sbuf_tensor([NL, CH * SS], _f32))
        emit1 = es.enter_context(nc.sbuf_tensor([NL, CH * SS], _f32))
        strip0 = es.enter_context(nc.sbuf_tensor([CH, SS], _f32))
        strip1 = es.enter_context(nc.sbuf_tensor([CH, SS], _f32))
        strip2 = es.enter_context(nc.sbuf_tensor([CH, SS], _f32))
        strip3 = es.enter_context(nc.sbuf_tensor([CH, SS], _f32))
        psum0 = es.enter_context(nc.psum_tensor([CH, SS], _f32))
        psum1 = es.enter_context(nc.psum_tensor([CH, SS], _f32))
        psum2 = es.enter_context(nc.psum_tensor([CH, SS], _f32))
        psum3 = es.enter_context(nc.psum_tensor([CH, SS], _f32))
        LbgWg = es.enter_context(nc.sbuf_tensor([NL, 2 * (S + 1)], _f32))
        Bt = es.enter_context(nc.sbuf_tensor([NL, S + 1], _f32))
        XU = es.enter_context(nc.sbuf_tensor([NL, 2 * (S + 1)], _f32))
        Yt = es.enter_context(nc.sbuf_tensor([NL, S], _f32))
        Ct = es.enter_context(nc.sbuf_tensor([NL, 1], _f32))
        snaps = es.enter_context(nc.sbuf_tensor([NL, T + 1], _f32))
        rs = es.enter_context(nc.sbuf_tensor([NL, T], _f32))
        s_oh = es.enter_context(nc.semaphore("s_oh"))
        s_mask = es.enter_context(nc.semaphore("s_mask"))
        s_lp = es.enter_context(nc.semaphore("s_lp"))
        s_pad = es.enter_context(nc.semaphore("s_pad"))
        s_exp = es.enter_context(nc.semaphore("s_exp"))
        s_scr = es.enter_context(nc.semaphore("s_scr"))
        s_pcT = es.enter_context(nc.semaphore("s_pcT"))
        s_mm = es.enter_context(nc.semaphore("s_mm"))
        s_cp = es.enter_context(nc.semaphore("s_cp"))
        s_em0 = es.enter_context(nc.semaphore("s_em0"))
        s_em1 = es.enter_context(nc.semaphore("s_em1"))
        s_em2 = es.enter_context(nc.semaphore("s_em2"))
        s_em3 = es.enter_context(nc.semaphore("s_em3"))
        s_out = es.enter_context(nc.semaphore("s_out"))
        s_vch = es.enter_context(nc.semaphore("s_vch"))
        s_vdone = es.enter_context(nc.semaphore("s_vdone"))
        block = es.enter_context(nc.Block())
        nats = [nat0, nat0]
        pads = [pad0, pad0]
        emits = [emit0, emit1]
        strips = [strip0, strip1, strip2, strip3]
        s_ems = [s_em0, s_em1, s_em2, s_em3]
        psums = [psum0, psum1, psum2, psum3]

        lw_view = LbgWg[:, :].rearrange("p (g j) -> p g j", g=2)
        xu_view = XU[:, :].rearrange("p (g j) -> p g j", g=2)
        b_rep = Bt[:, :].unsqueeze(1).broadcast_to((NL, 2, S + 1))
        y_rep = Yt[:, :].unsqueeze(1).broadcast_to((NL, 2, S))

        @block.sync
        def _(sync):
            sync.dma_start(oh_t[:], oh_in[:]).then_inc(s_oh, 16)
            sync.dma_start(masks[:], mk_in[:]).then_inc(s_mask, 16)
            sync.dma_start(nat0[:], lp_in[0:128, :]).then_inc(s_lp, 16)
            for q in range(4):
                sync.wait_ge(s_exp, q + 1)
                sync.dma_start(
                    scratch[q * 128 * NL:(q + 1) * 128 * NL, :]
                    .rearrange("(t n) c -> t (n c)", n=NL),
                    pads[q % 2][:],
                ).then_inc(s_scr, 16)
                if q + 1 <= 3:
                    sync.dma_start(
                        nat0[:], lp_in[(q + 1) * 128:(q + 2) * 128, :]
                    ).then_inc(s_lp, 16)
            sync.wait_ge(s_scr, 64)
            sync.dma_start_transpose(pcT[:], scratch[:]).then_inc(s_pcT, 16)
            for tc in range(NCH):
                if tc >= 2:
                    sync.wait_ge(s_vch, tc - 1)
                emit_t = emits[tc % 2]
                for n in range(NL):
                    idx = tc * NL + n
                    sync.wait_ge(s_cp, idx + 1)
                    sync.dma_start(
                        emit_t[n:n + 1, :].rearrange("o (t s) -> o t s", t=CH),
                        strips[idx % 4][:],
                    ).then_inc(s_ems[idx % 4], 16)
            sync.wait_ge(s_vdone, 1)
            sync.dma_start(snaps_out[:], snaps[:]).then_inc(s_out, 16)
            sync.dma_start(rs_out[:], rs[:]).then_inc(s_out, 16)

        @block.scalar
        def _(scalar):
            scalar.wait_ge(s_pad, 1)
            for q in range(4):
                scalar.wait_ge(s_lp, 16 * (q + 1))
                if q >= 1:
                    scalar.wait_ge(s_scr, 16 * q)
                scalar.activation(
                    pads[q % 2][:, :].rearrange("t (n c) -> t n c", n=NL)[:, :, 0:C],
                    nats[q % 2][:, :].rearrange("t (n c) -> t n c", n=NL),
                    _EXP,
                ).then_inc(s_exp, 1)
            for idx in range(NMM):
                scalar.wait_ge(s_mm, idx + 1)
                if idx >= 4:
                    scalar.wait_ge(s_ems[idx % 4], 16 * (idx // 4))
                scalar.copy(strips[idx % 4][:], psums[idx % 4][:]).then_inc(s_cp, 1)

        @block.tensor
        def _(tensor):
            tensor.wait_ge(s_pcT, 16)
            tensor.wait_ge(s_oh, 16)
            for tc in range(NCH):
                base0 = tc * CH * NL
                for n in range(NL):
                    idx = tc * NL + n
                    if idx >= 4:
                        tensor.wait_ge(s_cp, idx - 3)
                    tensor.matmul(
                        psums[idx % 4][:],
                        pcT[:, base0 + n: base0 + CH * NL: NL],
                        oh_t[:, n * SS:(n + 1) * SS],
                        start=True, stop=True,
                    ).then_inc(s_mm, 1)

        @block.vector
        def _(vector):
            vector.memset(pad0[:], 0.0).then_inc(s_pad, 1)
            vector.memset(LbgWg[:], 0.0)
            vector.memset(snaps[:], 0.0)
            vector.memset(rs[:], 0.0)
            vector.wait_ge(s_mask, 16)
            for s_e in s_ems:
                vector.wait_ge(s_e, 256)

            def opA():
                return vector.scalar_tensor_tensor(
                    xu_view, b_rep, 1.0, lw_view,
                    op0=_MUL, op1=_ADD, accum_out=Ct[:, :],
                )

            # init from emission block t=0
            em0 = emits[0][:, 0:SS]
            vector.tensor_scalar(Bt[:, :], masks[:, 0:S + 1],
                                 em0[:, 2 * S:2 * S + 1], None, op0=_MUL)
            vector.tensor_tensor(LbgWg[:, 1:S + 1], masks[:, S + 1:SS],
                                 em0[:, 0:S], op=_MUL)
            vector.tensor_tensor(LbgWg[:, S + 2:2 * S + 2], masks[:, S + 1:SS],
                                 em0[:, S:2 * S], op=_MUL)

            for tc in range(NCH):
                if tc >= 1:
                    for s_e in s_ems:
                        vector.wait_ge(s_e, 256 * (tc + 1))
                emit_t = emits[tc % 2]
                last = None
                for tl in range(CH):
                    t = tc * CH + tl
                    if t == 0:
                        continue
                    em = emit_t[:, tl * SS:(tl + 1) * SS]
                    opA()
                    vector.tensor_copy(snaps[:, t:t + 1], XU[:, S:S + 1])
                    vector.reciprocal(rs[:, t - 1:t], Ct[:, :])
                    vector.tensor_tensor(Yt[:, :], LbgWg[:, 1:S + 1],
                                         XU[:, S + 1:2 * S + 1], op=_ADD)
                    vector.scalar_tensor_tensor(
                        lw_view[:, :, 1:S + 1], y_rep, rs[:, t - 1:t],
                        em[:, 0:2 * S].rearrange("p (g j) -> p g j", g=2),
                        op0=_MUL, op1=_MUL,
                    )
                    last = vector.tensor_scalar(
                        Bt[:, :], XU[:, 0:S + 1],
                        em[:, 2 * S:2 * S + 1], rs[:, t - 1:t],
                        op0=_MUL, op1=_MUL,
                    )
                last.then_inc(s_vch, 1)
            opA()
            vector.tensor_copy(snaps[:, T:T + 1], XU[:, S:S + 1]).then_inc(s_vdone, 1)

    return nc


def _get_program():
    global _prog_cache
    if _prog_cache is None:
        _prog_cache = _build_program()
    return _prog_cache


def _host_prep(targets, target_lengths):
    tl = target_lengths.astype(np.int64)
    delta = S - tl                                     # (N,) in [0, 64]
    jj = np.arange(S)[None, :]
    src_idx = jj - delta[:, None]
    lab = np.where(
        src_idx >= 0,
        np.take_along_axis(targets.astype(np.int64),
                           np.clip(src_idx, 0, S - 1), axis=1),
        0,
    )                                                  # (N, S) right-aligned labels
    kmask = np.zeros((N, S), bool)
    kmask[:, 1:] = lab[:, 1:] != lab[:, :-1]
    kmask &= jj > delta[:, None]                       # no skip into first real label
    kshift = np.zeros((N, S), np.float32)
    kshift[:, :-1] = kmask[:, 1:]

    oh = np.zeros((N, SS, CP), np.float32)
    np.put_along_axis(oh[:, 0:S, :], lab[:, :, None], 1.0, axis=2)
    plk_part = np.zeros((N, S, CP), np.float32)
    np.put_along_axis(plk_part, lab[:, :, None], kshift[:, :, None], axis=2)
    oh[:, S:2 * S, :] = plk_part
    oh[:, 2 * S, 0] = 1.0
    oh_t = np.ascontiguousarray(oh.transpose(2, 0, 1)).astype(ml_dtypes.float8_e4m3)

    masks = np.zeros((N, SS), np.float32)
    nn = np.arange(N)
    masks[nn, delta] = 1.0                             # B init at j = delta
    masks[nn, S + 1 + delta] = 1.0                     # Lb init at j = delta
    return oh_t, masks


def kernel(log_probs, targets, input_lengths, target_lengths):
    lp = np.ascontiguousarray(np.asarray(log_probs, np.float32))
    targets = np.asarray(targets, np.int32)
    il = np.asarray(input_lengths).astype(np.int64)
    tl = np.asarray(target_lengths, np.int32)

    oh_t, masks = _host_prep(targets, tl.astype(np.int64))
    nc = _get_program()

    in_maps = []
    for k in range(NCORES):
        nsl = slice(k * NL, (k + 1) * NL)
        in_maps.append({
            "lp": np.ascontiguousarray(lp[:, nsl, :]).reshape(T, NL * C),
            "oh": np.ascontiguousarray(oh_t[:, nsl, :]).reshape(CP, NL * SS),
            "masks": np.ascontiguousarray(masks[nsl]),
        })
    res = run_bass_kernel_spmd(nc, in_maps, list(range(NCORES))).results

    loss = np.zeros(N, np.float64)
    nloc = np.arange(NL)
    for k in range(NCORES):
        snaps = res[k]["snaps"].astype(np.float64)     # (NL, T+1)
        rsv = res[k]["rs"].astype(np.float64)          # (NL, T)
        with np.errstate(divide="ignore"):
            logr = np.log(rsv[:, 0:T - 1])             # r_u at col u-1, u = 1..T-1
        cum = np.concatenate([np.zeros((NL, 1)), np.cumsum(logr, axis=1)], axis=1)
        tstar = il[k * NL:(k + 1) * NL] - 1
        v = snaps[nloc, tstar + 1]
        with np.errstate(divide="ignore", invalid="ignore"):
            lo = -(np.log(v) - cum[nloc, tstar])
        lo = np.where(np.isfinite(lo) & (lo < 1e10), lo, 0.0)
        loss[k * NL:(k + 1) * NL] = lo
    return loss.astype(np.float32)
